# revision 43
# baseline (speedup 1.0000x reference)
"""Multi-head causal attention (B=2, T=2048, C=1024, H=16, HS=64) on 8 TRN2
NeuronCores.

Sharding: (batch, head-group) grid — core c handles batch c//4 and heads
4*(c%4)..4*(c%4)+3. Each core receives its batch's pre-transposed
activations xT [C, T] in bf16, its 4 heads' QKV weight slices packed
[128, 8, 256], and its 256-row slice of w_proj^T packed [128, 2, C]. Each
core computes a partial output [T, C] in bf16; the host sums 4 partials
per batch and adds b_proj.

Per-core kernel (all matmuls bf16):
  - The 4 heads are processed as two head-pairs p=0,1, giving two
    independent attention pipelines (PSUM fits exactly two).
  - QT/KT [128(2 heads x 64), T] per pair via lhsT=weight chunks, rhs=xT.
  - V computed in NATURAL layout (no PE transpose): lhsT=xT chunk
    [128c,128t], rhs=wv chunk [128c,256] -> V [t, 4*64]; copied into
    vaug[pair] [P, NKB, 2, 128] slots [V_h | ones], ones via gpsimd
    memset (no ones/ident DMA, no transposes).
  - Flash-style causal attention in transposed layout: S^T[keys, q]
    blocks via lhsT=KT block, rhs=QT slice; exp on ScalarE (no max
    subtraction -- scores are O(1) by construction); O^T = [V_h|1].T @
    P^T accumulated over key blocks gives both O rows (0:64) and the
    softmax sums l (64:128).
  - Causal masking of diagonal blocks via bf16 tri multiply on GpSimd
    (frees DVE).
  - Normalize directly from PSUM: reciprocal_approx_fast(l) and
    O*rinv read otps in place (no staging copies).
  - proj contracts both pairs' ohat against wpt halves; for the LAST
    group the pair-1 half is precomputed into an f32 SBUF partial during
    pair-0's attention, so the tail is only pair-0 matmuls + adds.

Scheduling: the PE p-state clock reaches 2.4 GHz only under sustained
activity, so the emission (a) interleaves the two head-pairs' attention
groups (last two reversed: (1,3) before (0,3)), (b) pops "filler" PE
units (QKV/V chains, proj chunks) into every attention jg slot, and
(c) skews O^T one jg behind S^T/exp so the PE never waits on ScalarE.
Startup DMAs are chained in a sliding window ordered by first use so
the first QKV chain starts ~3us in.
"""

import sys
from contextlib import ExitStack

if "/opt/trn_rl_repo" not in sys.path:
    sys.path.insert(0, "/opt/trn_rl_repo")

import numpy as np

import concourse.mybir as mybir
import concourse.tile as tile
from concourse import bacc
from concourse.bass import ts
from concourse.bass_utils import run_bass_kernel_spmd
from concourse.tile_rust import add_dep_helper

B, T, C = 2, 2048, 1024
H, HS = 16, 64
NCORES = 8
HPC = 4  # heads per core
P = 128
G = 512  # q-group size
NG = T // G
KB = 128  # key block
NKB = T // KB
NPO = C // P  # contraction chunks
F32 = mybir.dt.float32
BF16 = mybir.dt.bfloat16
SCALE = float(HS) ** -0.5

_nc_cache = {}

# HW-risk bisection flags
TRI_GPSIMD = False   # tri-mask multiply on GpSimd (else DVE)
ONES_GPSIMD = False  # vaug ones memset on GpSimd (else DVE)
DIRECT_NORM = False  # normalize reading otps PSUM in place (else staged)


def _emit(tc):
    nc = tc.nc
    xt = nc.dram_tensor("xt", [C, T], BF16, kind="ExternalInput").ap()
    wqd = nc.dram_tensor("wq2", [P, NPO, 256], BF16, kind="ExternalInput").ap()
    wkd = nc.dram_tensor("wk2", [P, NPO, 256], BF16, kind="ExternalInput").ap()
    wvd = nc.dram_tensor("wv2", [P, NPO, 256], BF16, kind="ExternalInput").ap()
    wptd = nc.dram_tensor("wpt", [P, 2, C], BF16, kind="ExternalInput").ap()
    trid = nc.dram_tensor("tri", [P, P], BF16, kind="ExternalInput").ap()
    out = nc.dram_tensor("out", [T, C], BF16, kind="ExternalOutput").ap()
    # pair-1's last-group (g0) proj partial, summed into out rows 0:512 by
    # the host; decouples the two pairs so the tail is pair-0 only
    outp1 = nc.dram_tensor("outp1", [G, C], BF16, kind="ExternalOutput").ap()

    ctx = ExitStack()
    persist = ctx.enter_context(tc.tile_pool(name="persist", bufs=1))
    xt_pool = ctx.enter_context(tc.tile_pool(name="xtp", bufs=1))
    qk_pool = ctx.enter_context(tc.tile_pool(name="qkp", bufs=2))
    vaug_pool = ctx.enter_context(tc.tile_pool(name="vaugp", bufs=2))
    pt_pool = ctx.enter_context(tc.tile_pool(name="ptp", bufs=6))
    norm_pool = ctx.enter_context(tc.tile_pool(name="normp", bufs=3))
    ohat_pool = ctx.enter_context(tc.tile_pool(name="ohatp", bufs=2))
    out_pool = ctx.enter_context(tc.tile_pool(name="outp", bufs=4))
    st_psum = ctx.enter_context(tc.tile_pool(name="stps", bufs=2, space="PSUM"))
    ot_psum = ctx.enter_context(tc.tile_pool(name="otps", bufs=2, space="PSUM"))
    mm_psum = ctx.enter_context(tc.tile_pool(name="mmps", bufs=2, space="PSUM"))

    wq_sb = persist.tile([P, NPO, 256], BF16, tag="wq")
    wk_sb = persist.tile([P, NPO, 256], BF16, tag="wk")
    wv_sb = persist.tile([P, NPO, 256], BF16, tag="wv")
    wpt_sb = persist.tile([P, 2, C], BF16, tag="wpt")
    tri_sb = persist.tile([P, P], BF16, tag="tri")
    xtt = xt_pool.tile([P, NPO, T], BF16, tag="xt", name="xtt")

    # ---- startup DMAs: sliding-window chained, ordered by first use ----
    # sync queue: xt pieces; gpsimd queue: weights. Window of 2 per queue
    # so the front of each queue gets full ring bandwidth.
    xt_src = xt.rearrange("(pi po) t -> pi po t", po=NPO)
    sync_dmas = []
    gp_dmas = []

    def sdma(dst, src):
        i = nc.sync.dma_start(dst, src)
        if len(sync_dmas) >= 4:
            add_dep_helper(i.ins, sync_dmas[-4].ins, sync=True)
        sync_dmas.append(i)

    def gdma(dst, src):
        i = nc.gpsimd.dma_start(dst, src)
        if len(gp_dmas) >= 4:
            add_dep_helper(i.ins, gp_dmas[-4].ins, sync=True)
        gp_dmas.append(i)

    # first-use order: q/k chains (wq, wk, xt t0:512) -> V tb0-3 (wv) ->
    # rest of xt -> tri -> wpt. Window-4 chains keep the front prioritized
    # without round-trip serialization.
    gdma(wq_sb[:, 0:4, :], wqd[:, 0:4, :])
    sdma(xtt[:, 0:4, 0:512], xt_src[:, 0:4, 0:512])
    gdma(wk_sb[:, 0:4, :], wkd[:, 0:4, :])
    sdma(xtt[:, 4:8, 0:512], xt_src[:, 4:8, 0:512])
    gdma(wq_sb[:, 4:8, :], wqd[:, 4:8, :])
    gdma(wk_sb[:, 4:8, :], wkd[:, 4:8, :])
    gdma(wv_sb[:, 0:4, :], wvd[:, 0:4, :])
    gdma(wv_sb[:, 4:8, :], wvd[:, 4:8, :])
    sdma(xtt[:, 0:4, 512:1024], xt_src[:, 0:4, 512:1024])
    sdma(xtt[:, 4:8, 512:1024], xt_src[:, 4:8, 512:1024])
    gdma(tri_sb[:], trid[:])
    sdma(xtt[:, 0:4, 1024:2048], xt_src[:, 0:4, 1024:2048])
    sdma(xtt[:, 4:8, 1024:2048], xt_src[:, 4:8, 1024:2048])
    gdma(wpt_sb[:], wptd[:])

    def new_state(p):
        st = {
            "p": p,
            "qt": qk_pool.tile([P, T], BF16, tag="qt", name=f"qt{p}"),
            "kt": qk_pool.tile([P, T], BF16, tag="kt", name=f"kt{p}"),
            "ohat": ohat_pool.tile([P, T], BF16, tag="ohat", name=f"oh{p}"),
            # vaug[pair]: per key block j, 4 slots of 64 cols:
            # [V_h0 | ones | ones | V_h1]. O^T lhsT for h0 = slots 0:2
            # ([V|1] -> O rows 0:64, l rows 64:128), for h1 = slots 2:4
            # ([1|V] -> l rows 0:64, O rows 64:128) so both norm mults are
            # same-partition-base reads from PSUM.
            "vaug": vaug_pool.tile(
                [P, NKB, 4, 64], BF16, tag="vaug", name=f"va{p}"
            ),
        }
        eng = nc.gpsimd if ONES_GPSIMD else nc.vector
        eng.memset(st["vaug"][:, :, 1:3, :], 1.0)
        return st

    # total exps = 2 heads x 2 pairs x sum_g(2g+2) = 80
    phase = {"exps_left": 80, "flip": 0}

    # ---------- building blocks ----------
    def emit_qk_group(st, which, tg, copy_eng):
        w_sb, dst = {
            "q": (wq_sb, st["qt"]),
            "k": (wk_sb, st["kt"]),
        }[which]
        p = st["p"]
        ps = mm_psum.tile([P, 512], F32, tag="mm", name=f"qk{which}{tg}{p}")
        for po in range(NPO):
            nc.tensor.matmul(
                ps[:],
                w_sb[:, po, 128 * p : 128 * p + 128],
                xtt[:, po, ts(tg, 512)],
                start=(po == 0),
                stop=(po == NPO - 1),
            )
        if copy_eng == "scalar":
            nc.scalar.copy(dst[:, ts(tg, 512)], ps[:])
        else:
            nc.vector.tensor_copy(dst[:, ts(tg, 512)], ps[:])

    def emit_v_block(tb):
        # V natural layout for key block tb, all 4 heads at once
        vb = mm_psum.tile([P, 2, 2, 64], F32, tag="mm", name=f"vb{tb}")
        for po in range(NPO):
            nc.tensor.matmul(
                vb[:],
                xtt[:, po, ts(tb, KB)],
                wv_sb[:, po, :],
                start=(po == 0),
                stop=(po == NPO - 1),
            )
        for p in range(2):
            # V_h0 -> slot 0, V_h1 -> slot 3 (stride-3 step slice)
            nc.vector.tensor_copy(
                sts[p]["vaug"][:, tb, 0:4:3, :], vb[:, p, :, :]
            )

    def emit_proj_chunk(g, tc4, copy_eng):
        t0 = G * g + P * tc4
        o_sb = out_pool.tile([P, C], BF16, tag="osb", name=f"osb{g}{tc4}")
        for n in range(C // 512):
            pj = mm_psum.tile([P, 512], F32, tag="mm", name=f"pj{n}")
            nc.tensor.matmul(
                pj[:],
                sts[0]["ohat"][:, t0 : t0 + P],
                wpt_sb[:, 0, ts(n, 512)],
                start=True,
                stop=False,
            )
            nc.tensor.matmul(
                pj[:],
                sts[1]["ohat"][:, t0 : t0 + P],
                wpt_sb[:, 1, ts(n, 512)],
                start=False,
                stop=True,
            )
            eng = copy_eng
            if eng == "auto":
                # ScalarE is saturated by exps until the attention tail
                if phase["exps_left"] > 0:
                    eng = "vector"
                else:
                    phase["flip"] ^= 1
                    eng = "scalar" if phase["flip"] else "vector"
            if eng == "scalar":
                nc.scalar.copy(o_sb[:, ts(n, 512)], pj[:])
            else:
                nc.vector.tensor_copy(o_sb[:, ts(n, 512)], pj[:])
        nc.sync.dma_start(out[t0 : t0 + P, :], o_sb[:])

    # last-group (g0) proj: pairs fully decoupled. pair-1 -> outp1 (filler
    # during (0,0)); pair-0 tail uses the then-idle ScalarE for half its
    # copies and alternates mm/ot psum pools for a 4-deep pipeline.
    def emit_proj_g0_p1(tc4):
        t0 = P * tc4
        o_sb = out_pool.tile([P, C], BF16, tag="osb", name=f"o01{tc4}")
        for n in range(C // 512):
            pj = mm_psum.tile([P, 512], F32, tag="mm", name=f"pg0a{tc4}{n}")
            nc.tensor.matmul(
                pj[:],
                sts[1]["ohat"][:, t0 : t0 + P],
                wpt_sb[:, 1, ts(n, 512)],
                start=True,
                stop=True,
            )
            nc.vector.tensor_copy(o_sb[:, ts(n, 512)], pj[:])
        nc.sync.dma_start(outp1[P * tc4 : P * tc4 + P, :], o_sb[:])

    def emit_proj_g0_p0(tc4):
        t0 = P * tc4
        o_sb = out_pool.tile([P, C], BF16, tag="osb", name=f"osb0{tc4}")
        for n in range(C // 512):
            pool = ot_psum if tc4 % 2 else mm_psum
            tag = "ot" if tc4 % 2 else "mm"
            pj = pool.tile([P, 512], F32, tag=tag, name=f"pg0b{tc4}{n}")
            nc.tensor.matmul(
                pj[:],
                sts[0]["ohat"][:, t0 : t0 + P],
                wpt_sb[:, 0, ts(n, 512)],
                start=True,
                stop=True,
            )
            if n == 0:
                nc.scalar.copy(o_sb[:, ts(n, 512)], pj[:])
            else:
                nc.vector.tensor_copy(o_sb[:, ts(n, 512)], pj[:])
            nc.sync.dma_start(
                out[t0 : t0 + P, ts(n, 512)], o_sb[:, ts(n, 512)]
            )

    # ---------- filler unit queue ----------
    # each unit: (force_key, release_key, fn): forced (emitted) at order
    # index force_key; poppable as filler once the current order index
    # >= release_key. Release gating reserves PE work for the late,
    # scalar-bound groups.
    BIG = 99
    units = []
    cur_idx = [0]

    def pop_units(maxn):
        n = 0
        i = 0
        while i < len(units) and n < maxn:
            if units[i][1] <= cur_idx[0]:
                _, _, fn = units.pop(i)
                fn()
                n += 1
            else:
                i += 1

    def force_units(idx):
        i = 0
        while i < len(units):
            if units[i][0] <= idx:
                _, _, fn = units.pop(i)
                fn()
            else:
                i += 1

    # ---------- attention for one (pair, g) with one-jg S/exp -> O skew ----
    def emit_attn_g(st, g, fine_norm=False):
        p, qt, kt, vaug, ohat = st["p"], st["qt"], st["kt"], st["vaug"], st["ohat"]
        n_j = 4 * g + 4
        n_jg = n_j // 2
        otps_h = [
            ot_psum.tile([P, G], F32, tag="ot", name=f"ot{p}{g}{h}")
            for h in range(2)
        ]
        pend = None  # (js, pt_h) waiting for O^T

        def emit_s_exp(jg):
            js = (2 * jg, 2 * jg + 1)
            stps_h = [
                st_psum.tile([P, 2, G], F32, tag="st", name=f"st{p}{g}{h}")
                for h in range(2)
            ]
            pt_h = [
                pt_pool.tile([P, 2, G], BF16, tag=f"pt{h}", name=f"pt{p}{g}{h}")
                for h in range(2)
            ]
            # both blocks' S matmuls write [qmin:G] (the 2nd diagonal
            # block computes 128 extra masked cols) so ONE exp per (h, jg)
            # reads only initialized PSUM; O^T still reads [q0:G] per block
            qmin = max(0, 128 * (js[0] - 4 * g))
            for idx, j in enumerate(js):
                for h in range(2):
                    hb = 64 * h
                    nc.tensor.matmul(
                        stps_h[h][:, idx, qmin:G],
                        kt[hb : hb + 64, ts(j, KB)],
                        qt[hb : hb + 64, G * g + qmin : G * (g + 1)],
                        start=True,
                        stop=True,
                    )
            for h in range(2):
                nc.scalar.activation(
                    pt_h[h][:, :, qmin:G],
                    stps_h[h][:, :, qmin:G],
                    mybir.ActivationFunctionType.Exp,
                    scale=SCALE,
                )
            phase["exps_left"] -= 2
            # causal mask on the diagonal boundary blocks (on GpSimd; the
            # one-jg S/exp->O skew gives this slack)
            for idx, j in enumerate(js):
                r = j - 4 * g
                if r >= 0:
                    q0 = 128 * r
                    teng = nc.gpsimd if TRI_GPSIMD else nc.vector
                    for h in range(2):
                        teng.tensor_tensor(
                            pt_h[h][:, idx, q0 : q0 + 128],
                            pt_h[h][:, idx, q0 : q0 + 128],
                            tri_sb[:],
                            mybir.AluOpType.mult,
                        )
            return (js, pt_h)

        def emit_o(pend):
            js, pt_h = pend
            for idx, j in enumerate(js):
                r = j - 4 * g
                q0 = 128 * r if r >= 0 else 0
                for h in range(2):
                    nc.tensor.matmul(
                        otps_h[h][:, q0:G],
                        vaug[:, j, 2 * h : 2 * h + 2, :],
                        pt_h[h][:, idx, q0:G],
                        start=(j == 0),
                        stop=(j == n_j - 1),
                    )

        for jg in range(n_jg + 1):
            if jg < n_jg:
                pend_new = emit_s_exp(jg)
                # extra pops in the last slots cover the scalar drain at
                # the group boundary
                pop_units(3 if jg >= n_jg - 2 else 2)
            if pend is not None:
                emit_o(pend)
            pend = pend_new if jg < n_jg else None

        # normalize: otps_h[0] = [O_h0 (0:64) | l_h0 (64:128)],
        # otps_h[1] = [l_h1 (0:64) | O_h1 (64:128)]. The l gather uses
        # partition-shifted copies (proven on HW); recip and the O*rinv
        # mults are all same-partition-base (mults read PSUM in place).
        l_sb = norm_pool.tile([P, G], F32, tag="lsb", name=f"l{p}{g}")
        rinv = norm_pool.tile([P, G], F32, tag="rinv", name=f"r{p}{g}")
        # fine_norm: 128-col pieces so dependent proj chunks can start
        # after the first piece instead of the whole-group norm
        npc = 4 if fine_norm else 1
        w = G // npc
        for pc in range(npc):
            c0, c1 = w * pc, w * pc + w
            nc.vector.tensor_copy(l_sb[0:64, c0:c1], otps_h[0][64:128, c0:c1])
            nc.vector.tensor_copy(l_sb[64:128, c0:c1], otps_h[1][0:64, c0:c1])
            nc.vector.reciprocal_approx_fast(rinv[:, c0:c1], l_sb[:, c0:c1])
            nc.vector.tensor_tensor(
                ohat[0:64, G * g + c0 : G * g + c1],
                otps_h[0][0:64, c0:c1], rinv[0:64, c0:c1],
                mybir.AluOpType.mult,
            )
            nc.vector.tensor_tensor(
                ohat[64:128, G * g + c0 : G * g + c1],
                otps_h[1][64:128, c0:c1], rinv[64:128, c0:c1],
                mybir.AluOpType.mult,
            )
        pop_units(3)

    # ================= emission =================
    sts = {0: new_state(0), 1: new_state(1)}

    # attention interleave order: the fat g3 groups run mid-schedule where
    # filler supply is plentiful; the tiny g0 groups finish, so the end
    # phase has the smallest exp load and the shortest norm->proj tail
    order = [(0, 1), (1, 1), (0, 2), (1, 2), (0, 3), (1, 3), (1, 0), (0, 0)]
    oidx = {pg: i for i, pg in enumerate(order)}

    # warmup: tg0 chains consume the first xt piece while tg1 lands, then
    # the first group's tg1 chains and V blocks 0-7 (needed by (0,1))
    emit_qk_group(sts[0], "q", 0, "scalar")
    emit_qk_group(sts[0], "k", 0, "vector")
    emit_qk_group(sts[0], "q", 1, "scalar")
    emit_qk_group(sts[0], "k", 1, "vector")
    for tb in range(8):
        emit_v_block(tb)

    # queue fillers: forced at the order index where required, poppable
    # one group earlier (just-in-time, preserving late-group reserve).
    # q(p,tg) is read only by group (p,tg); k(p,tg) by every (p,g>=tg).
    def queue_qk(p, tg, which):
        if which == "q":
            req = oidx[(p, tg)]
        else:
            req = min(oidx[(p, g)] for g in range(tg, NG))
        units.append(
            (req, max(0, req - 1),
             lambda p=p, w=which, tg=tg: emit_qk_group(
                 sts[p], w, tg,
                 "scalar" if phase["exps_left"] > 56 else "vector"))
        )

    for tg in range(NG):
        for p in (0, 1):
            for which in ("q", "k"):
                if p == 0 and tg in (0, 1):
                    continue  # emitted eagerly in warmup
                queue_qk(p, tg, which)
    for tb in range(8, NKB):
        g = tb // 4
        req = min(oidx[(0, g)], oidx[(1, g)])
        units.append((req, max(0, req - 1), lambda tb=tb: emit_v_block(tb)))
    units.sort(key=lambda u: u[0])

    for i, (p, g) in enumerate(order):
        cur_idx[0] = i
        force_units(i)
        emit_attn_g(sts[p], g, fine_norm=((p, g) == (0, 0)))
        done = {pg for pg in order[: i + 1]}
        if (0, g) in done and (1, g) in done and g > 0:
            # chained proj for g1-g3; two g3 chunks held to the drain to
            # bridge the (0,0)-norm -> g0-tail handoff at full clock
            for tc4 in range(G // P):
                rel = BIG if (g == 3 and tc4 >= 2) else i
                units.append(
                    (BIG, rel,
                     lambda g=g, tc4=tc4: emit_proj_chunk(g, tc4, "auto"))
                )
        if (p, g) == (1, 0):
            for tc4 in range(G // P):
                units.append((BIG, i, lambda tc4=tc4: emit_proj_g0_p1(tc4)))

    # tail: drain remaining fillers (incl. the two held g3 proj chunks),
    # then pair-0 g0 proj
    while units:
        _, _, fn = units.pop(0)
        fn()
    for tc4 in range(G // P):
        emit_proj_g0_p0(tc4)
    ctx.close()


def _build():
    if "nc" in _nc_cache:
        return _nc_cache["nc"]
    nc = bacc.Bacc("TRN2", target_bir_lowering=False, debug=False)
    with tile.TileContext(nc) as tc:
        _emit(tc)
    nc.compile()
    _nc_cache["nc"] = nc
    return nc


def _make_in_maps(x, wq, wk, wv, w_proj):
    import ml_dtypes

    bf16 = ml_dtypes.bfloat16
    xtb = [np.ascontiguousarray(x[b].T).astype(bf16) for b in range(B)]
    tri = np.triu(np.ones((P, P), dtype=np.float32)).astype(bf16)
    in_maps = []
    for c in range(NCORES):
        b, hg = c // 4, c % 4
        h0 = HPC * hg
        wq_cat = np.concatenate([wq[h0 + i] for i in range(HPC)], axis=1)
        wk_cat = np.concatenate([wk[h0 + i] for i in range(HPC)], axis=1)
        wv_cat = np.concatenate([wv[h0 + i] for i in range(HPC)], axis=1)
        wpt = w_proj[:, 256 * hg : 256 * (hg + 1)].T  # [256, C]
        in_maps.append(
            {
                "xt": xtb[b],
                "wq2": np.ascontiguousarray(
                    wq_cat.reshape(P, NPO, 256)).astype(bf16),
                "wk2": np.ascontiguousarray(
                    wk_cat.reshape(P, NPO, 256)).astype(bf16),
                "wv2": np.ascontiguousarray(
                    wv_cat.reshape(P, NPO, 256)).astype(bf16),
                "wpt": np.ascontiguousarray(
                    wpt.reshape(2, P, C).transpose(1, 0, 2)).astype(bf16),
                "tri": tri,
            }
        )
    return in_maps


def kernel(x, wq, wk, wv, w_proj, b_proj):
    x = np.asarray(x, dtype=np.float32)
    wq = np.asarray(wq, dtype=np.float32)
    wk = np.asarray(wk, dtype=np.float32)
    wv = np.asarray(wv, dtype=np.float32)
    w_proj = np.asarray(w_proj, dtype=np.float32)
    b_proj = np.asarray(b_proj, dtype=np.float32)

    nc = _build()
    in_maps = _make_in_maps(x, wq, wk, wv, w_proj)
    res = run_bass_kernel_spmd(nc, in_maps, core_ids=list(range(NCORES)))
    acc = np.zeros((B, T, C), dtype=np.float64)
    for c, r in enumerate(res.results):
        acc[c // 4] += np.asarray(r["out"], dtype=np.float64)
        acc[c // 4, 0:G] += np.asarray(r["outp1"], dtype=np.float64)
    return (acc + b_proj).astype(np.float32)


# revision 50
# speedup vs baseline: 1.0723x; 1.0723x over previous
"""Multi-head causal attention (B=2, T=2048, C=1024, H=16, HS=64) on 8 TRN2
NeuronCores.

Sharding: (batch, head-group) grid — core c handles batch c//4 and heads
4*(c%4)..4*(c%4)+3. Each core receives its batch's pre-transposed
activations xT [C, T] in bf16, its 4 heads' QKV weight slices packed
[128, 8, 256], and its 256-row slice of w_proj^T packed [128, 2, C]. Each
core computes a partial output [T, C] in bf16; the host sums 4 partials
per batch and adds b_proj.

Per-core kernel (all matmuls bf16):
  - The 4 heads are processed as two head-pairs p=0,1, giving two
    independent attention pipelines (PSUM fits exactly two).
  - QT/KT [128(2 heads x 64), T] per pair via lhsT=weight chunks, rhs=xT.
  - V computed in NATURAL layout (no PE transpose): lhsT=xT chunk
    [128c,128t], rhs=wv chunk [128c,256] -> V [t, 4*64]; copied into
    vaug[pair] [P, NKB, 2, 128] slots [V_h | ones], ones via gpsimd
    memset (no ones/ident DMA, no transposes).
  - Flash-style causal attention in transposed layout: S^T[keys, q]
    blocks via lhsT=KT block, rhs=QT slice; exp on ScalarE (no max
    subtraction -- scores are O(1) by construction); O^T = [V_h|1].T @
    P^T accumulated over key blocks gives both O rows (0:64) and the
    softmax sums l (64:128).
  - Causal masking of diagonal blocks via bf16 tri multiply on GpSimd
    (frees DVE).
  - Normalize directly from PSUM: reciprocal_approx_fast(l) and
    O*rinv read otps in place (no staging copies).
  - proj contracts both pairs' ohat against wpt halves; for the LAST
    group the pair-1 half is precomputed into an f32 SBUF partial during
    pair-0's attention, so the tail is only pair-0 matmuls + adds.

Scheduling: the PE p-state clock reaches 2.4 GHz only under sustained
activity, so the emission (a) interleaves the two head-pairs' attention
groups (last two reversed: (1,3) before (0,3)), (b) pops "filler" PE
units (QKV/V chains, proj chunks) into every attention jg slot, and
(c) skews O^T one jg behind S^T/exp so the PE never waits on ScalarE.
Startup DMAs are chained in a sliding window ordered by first use so
the first QKV chain starts ~3us in.
"""

import sys
from contextlib import ExitStack

if "/opt/trn_rl_repo" not in sys.path:
    sys.path.insert(0, "/opt/trn_rl_repo")

import numpy as np

import concourse.mybir as mybir
import concourse.tile as tile
from concourse import bacc
from concourse.bass import ts
from concourse.bass_utils import run_bass_kernel_spmd
from concourse.tile_rust import add_dep_helper

B, T, C = 2, 2048, 1024
H, HS = 16, 64
NCORES = 8
HPC = 4  # heads per core
P = 128
G = 512  # q-group size
NG = T // G
KB = 128  # key block
NKB = T // KB
NPO = C // P  # contraction chunks
F32 = mybir.dt.float32
BF16 = mybir.dt.bfloat16
SCALE = float(HS) ** -0.5

_nc_cache = {}

# HW-risk bisection flags
TRI_GPSIMD = False   # tri-mask multiply on GpSimd (else DVE)
ONES_GPSIMD = False  # vaug ones memset on GpSimd (else DVE)
DIRECT_NORM = False  # normalize reading otps PSUM in place (else staged)


def _emit(tc):
    nc = tc.nc
    xt = nc.dram_tensor("xt", [C, T], BF16, kind="ExternalInput").ap()
    wqd = nc.dram_tensor("wq2", [P, NPO, 256], BF16, kind="ExternalInput").ap()
    wkd = nc.dram_tensor("wk2", [P, NPO, 256], BF16, kind="ExternalInput").ap()
    wvd = nc.dram_tensor("wv2", [P, NPO, 256], BF16, kind="ExternalInput").ap()
    wptd = nc.dram_tensor("wpt", [P, 2, C], BF16, kind="ExternalInput").ap()
    trid = nc.dram_tensor("tri", [P, P], BF16, kind="ExternalInput").ap()
    out = nc.dram_tensor("out", [T, C], BF16, kind="ExternalOutput").ap()
    # pair-1's last-group (g3) proj partial, summed into out rows 1536:2048
    # by the host; decouples the two pairs so the tail is pair-0 only
    outp1 = nc.dram_tensor("outp1", [G, C], BF16, kind="ExternalOutput").ap()

    ctx = ExitStack()
    persist = ctx.enter_context(tc.tile_pool(name="persist", bufs=1))
    xt_pool = ctx.enter_context(tc.tile_pool(name="xtp", bufs=1))
    qk_pool = ctx.enter_context(tc.tile_pool(name="qkp", bufs=2))
    vaug_pool = ctx.enter_context(tc.tile_pool(name="vaugp", bufs=2))
    pt_pool = ctx.enter_context(tc.tile_pool(name="ptp", bufs=6))
    norm_pool = ctx.enter_context(tc.tile_pool(name="normp", bufs=3))
    ohat_pool = ctx.enter_context(tc.tile_pool(name="ohatp", bufs=2))
    out_pool = ctx.enter_context(tc.tile_pool(name="outp", bufs=4))
    st_psum = ctx.enter_context(tc.tile_pool(name="stps", bufs=2, space="PSUM"))
    ot_psum = ctx.enter_context(tc.tile_pool(name="otps", bufs=2, space="PSUM"))
    mm_psum = ctx.enter_context(tc.tile_pool(name="mmps", bufs=2, space="PSUM"))

    wq_sb = persist.tile([P, NPO, 256], BF16, tag="wq")
    wk_sb = persist.tile([P, NPO, 256], BF16, tag="wk")
    wv_sb = persist.tile([P, NPO, 256], BF16, tag="wv")
    wpt_sb = persist.tile([P, 2, C], BF16, tag="wpt")
    tri_sb = persist.tile([P, P], BF16, tag="tri")
    xtt = xt_pool.tile([P, NPO, T], BF16, tag="xt", name="xtt")

    # ---- startup DMAs: sliding-window chained, ordered by first use ----
    # sync queue: xt pieces; gpsimd queue: weights. Window of 2 per queue
    # so the front of each queue gets full ring bandwidth.
    xt_src = xt.rearrange("(pi po) t -> pi po t", po=NPO)
    sync_dmas = []
    gp_dmas = []

    def sdma(dst, src):
        i = nc.sync.dma_start(dst, src)
        if len(sync_dmas) >= 4:
            add_dep_helper(i.ins, sync_dmas[-4].ins, sync=True)
        sync_dmas.append(i)

    def gdma(dst, src):
        i = nc.gpsimd.dma_start(dst, src)
        if len(gp_dmas) >= 4:
            add_dep_helper(i.ins, gp_dmas[-4].ins, sync=True)
        gp_dmas.append(i)

    # first-use order: q/k chains (wq, wk, xt t0:512) -> V tb0-3 (wv) ->
    # rest of xt -> tri -> wpt. Window-4 chains keep the front prioritized
    # without round-trip serialization.
    gdma(wq_sb[:, 0:4, :], wqd[:, 0:4, :])
    sdma(xtt[:, 0:4, 0:512], xt_src[:, 0:4, 0:512])
    gdma(wk_sb[:, 0:4, :], wkd[:, 0:4, :])
    sdma(xtt[:, 4:8, 0:512], xt_src[:, 4:8, 0:512])
    gdma(wq_sb[:, 4:8, :], wqd[:, 4:8, :])
    gdma(wk_sb[:, 4:8, :], wkd[:, 4:8, :])
    gdma(wv_sb[:, 0:4, :], wvd[:, 0:4, :])
    gdma(wv_sb[:, 4:8, :], wvd[:, 4:8, :])
    sdma(xtt[:, 0:4, 512:1024], xt_src[:, 0:4, 512:1024])
    sdma(xtt[:, 4:8, 512:1024], xt_src[:, 4:8, 512:1024])
    gdma(tri_sb[:], trid[:])
    sdma(xtt[:, 0:4, 1024:2048], xt_src[:, 0:4, 1024:2048])
    sdma(xtt[:, 4:8, 1024:2048], xt_src[:, 4:8, 1024:2048])
    gdma(wpt_sb[:], wptd[:])

    def new_state(p):
        st = {
            "p": p,
            "qt": qk_pool.tile([P, T], BF16, tag="qt", name=f"qt{p}"),
            "kt": qk_pool.tile([P, T], BF16, tag="kt", name=f"kt{p}"),
            "ohat": ohat_pool.tile([P, T], BF16, tag="ohat", name=f"oh{p}"),
            # vaug[pair]: per key block j, 4 slots of 64 cols:
            # [V_h0 | ones | ones | V_h1]. O^T lhsT for h0 = slots 0:2
            # ([V|1] -> O rows 0:64, l rows 64:128), for h1 = slots 2:4
            # ([1|V] -> l rows 0:64, O rows 64:128) so both norm mults are
            # same-partition-base reads from PSUM.
            "vaug": vaug_pool.tile(
                [P, NKB, 4, 64], BF16, tag="vaug", name=f"va{p}"
            ),
        }
        eng = nc.gpsimd if ONES_GPSIMD else nc.vector
        eng.memset(st["vaug"][:, :, 1:3, :], 1.0)
        return st

    # total exps = 2 heads x 2 pairs x sum_g(2g+2) = 80
    phase = {"exps_left": 80, "flip": 0}

    # ---------- building blocks ----------
    def emit_qk_group(st, which, tg, copy_eng):
        w_sb, dst = {
            "q": (wq_sb, st["qt"]),
            "k": (wk_sb, st["kt"]),
        }[which]
        p = st["p"]
        ps = mm_psum.tile([P, 512], F32, tag="mm", name=f"qk{which}{tg}{p}")
        for po in range(NPO):
            nc.tensor.matmul(
                ps[:],
                w_sb[:, po, 128 * p : 128 * p + 128],
                xtt[:, po, ts(tg, 512)],
                start=(po == 0),
                stop=(po == NPO - 1),
            )
        if copy_eng == "scalar":
            nc.scalar.copy(dst[:, ts(tg, 512)], ps[:])
        else:
            nc.vector.tensor_copy(dst[:, ts(tg, 512)], ps[:])

    def emit_v_block(tb):
        # V natural layout for key block tb, all 4 heads at once
        vb = mm_psum.tile([P, 2, 2, 64], F32, tag="mm", name=f"vb{tb}")
        for po in range(NPO):
            nc.tensor.matmul(
                vb[:],
                xtt[:, po, ts(tb, KB)],
                wv_sb[:, po, :],
                start=(po == 0),
                stop=(po == NPO - 1),
            )
        for p in range(2):
            # V_h0 -> slot 0, V_h1 -> slot 3 (stride-3 step slice)
            nc.vector.tensor_copy(
                sts[p]["vaug"][:, tb, 0:4:3, :], vb[:, p, :, :]
            )

    def emit_proj_chunk(g, tc4, copy_eng):
        t0 = G * g + P * tc4
        o_sb = out_pool.tile([P, C], BF16, tag="osb", name=f"osb{g}{tc4}")
        for n in range(C // 512):
            pj = mm_psum.tile([P, 512], F32, tag="mm", name=f"pj{n}")
            nc.tensor.matmul(
                pj[:],
                sts[0]["ohat"][:, t0 : t0 + P],
                wpt_sb[:, 0, ts(n, 512)],
                start=True,
                stop=False,
            )
            nc.tensor.matmul(
                pj[:],
                sts[1]["ohat"][:, t0 : t0 + P],
                wpt_sb[:, 1, ts(n, 512)],
                start=False,
                stop=True,
            )
            eng = copy_eng
            if eng == "auto":
                # ScalarE is saturated by exps until the attention tail
                if phase["exps_left"] > 0:
                    eng = "vector"
                else:
                    phase["flip"] ^= 1
                    eng = "scalar" if phase["flip"] else "vector"
            if eng == "scalar":
                nc.scalar.copy(o_sb[:, ts(n, 512)], pj[:])
            else:
                nc.vector.tensor_copy(o_sb[:, ts(n, 512)], pj[:])
        nc.sync.dma_start(out[t0 : t0 + P, :], o_sb[:])

    # last-group (g3) proj: pairs fully decoupled. pair-1 -> outp1 (filler
    # during (0,3)); pair-0 tail uses the then-idle ScalarE for half its
    # copies and alternates mm/ot psum pools for a 4-deep pipeline.
    def emit_proj_g3_p1(tc4):
        t0 = G * 3 + P * tc4
        o_sb = out_pool.tile([P, C], BF16, tag="osb", name=f"o01{tc4}")
        for n in range(C // 512):
            pj = mm_psum.tile([P, 512], F32, tag="mm", name=f"pg0a{tc4}{n}")
            nc.tensor.matmul(
                pj[:],
                sts[1]["ohat"][:, t0 : t0 + P],
                wpt_sb[:, 1, ts(n, 512)],
                start=True,
                stop=True,
            )
            nc.vector.tensor_copy(o_sb[:, ts(n, 512)], pj[:])
        nc.sync.dma_start(outp1[P * tc4 : P * tc4 + P, :], o_sb[:])

    def emit_proj_g3_p0(tc4):
        t0 = G * 3 + P * tc4
        o_sb = out_pool.tile([P, C], BF16, tag="osb", name=f"osb0{tc4}")
        for n in range(C // 512):
            pool = ot_psum if tc4 % 2 else mm_psum
            tag = "ot" if tc4 % 2 else "mm"
            pj = pool.tile([P, 512], F32, tag=tag, name=f"pg0b{tc4}{n}")
            nc.tensor.matmul(
                pj[:],
                sts[0]["ohat"][:, t0 : t0 + P],
                wpt_sb[:, 0, ts(n, 512)],
                start=True,
                stop=True,
            )
            if n == 0:
                nc.scalar.copy(o_sb[:, ts(n, 512)], pj[:])
            else:
                nc.vector.tensor_copy(o_sb[:, ts(n, 512)], pj[:])
            nc.sync.dma_start(
                out[t0 : t0 + P, ts(n, 512)], o_sb[:, ts(n, 512)]
            )

    # ---------- filler unit queue ----------
    # each unit: (force_key, release_key, fn): forced (emitted) at order
    # index force_key; poppable as filler once the current order index
    # >= release_key. Release gating reserves PE work for the late,
    # scalar-bound groups.
    BIG = 99
    units = []
    cur_idx = [0]

    def pop_units(maxn):
        n = 0
        i = 0
        while i < len(units) and n < maxn:
            if units[i][1] <= cur_idx[0]:
                _, _, fn = units.pop(i)
                fn()
                n += 1
            else:
                i += 1

    def force_units(idx):
        i = 0
        while i < len(units):
            if units[i][0] <= idx:
                _, _, fn = units.pop(i)
                fn()
            else:
                i += 1

    # ---------- attention for one (pair, g) with one-jg S/exp -> O skew ----
    def emit_attn_g(st, g, fine_norm=False):
        p, qt, kt, vaug, ohat = st["p"], st["qt"], st["kt"], st["vaug"], st["ohat"]
        n_j = 4 * g + 4
        n_jg = n_j // 2
        otps_h = [
            ot_psum.tile([P, G], F32, tag="ot", name=f"ot{p}{g}{h}")
            for h in range(2)
        ]
        pend = None  # (js, pt_h) waiting for O^T

        def emit_s_exp(jg):
            js = (2 * jg, 2 * jg + 1)
            stps_h = [
                st_psum.tile([P, 2, G], F32, tag="st", name=f"st{p}{g}{h}")
                for h in range(2)
            ]
            pt_h = [
                pt_pool.tile([P, 2, G], BF16, tag=f"pt{h}", name=f"pt{p}{g}{h}")
                for h in range(2)
            ]
            # both blocks' S matmuls write [qmin:G] (the 2nd diagonal
            # block computes 128 extra masked cols) so ONE exp per (h, jg)
            # reads only initialized PSUM; O^T still reads [q0:G] per block
            qmin = max(0, 128 * (js[0] - 4 * g))
            for idx, j in enumerate(js):
                for h in range(2):
                    hb = 64 * h
                    nc.tensor.matmul(
                        stps_h[h][:, idx, qmin:G],
                        kt[hb : hb + 64, ts(j, KB)],
                        qt[hb : hb + 64, G * g + qmin : G * (g + 1)],
                        start=True,
                        stop=True,
                    )
            for h in range(2):
                nc.scalar.activation(
                    pt_h[h][:, :, qmin:G],
                    stps_h[h][:, :, qmin:G],
                    mybir.ActivationFunctionType.Exp,
                    scale=SCALE,
                )
            phase["exps_left"] -= 2
            # causal mask on the diagonal boundary blocks (on GpSimd; the
            # one-jg S/exp->O skew gives this slack)
            for idx, j in enumerate(js):
                r = j - 4 * g
                if r >= 0:
                    q0 = 128 * r
                    teng = nc.gpsimd if TRI_GPSIMD else nc.vector
                    for h in range(2):
                        teng.tensor_tensor(
                            pt_h[h][:, idx, q0 : q0 + 128],
                            pt_h[h][:, idx, q0 : q0 + 128],
                            tri_sb[:],
                            mybir.AluOpType.mult,
                        )
            return (js, pt_h)

        def emit_o(pend):
            js, pt_h = pend
            for idx, j in enumerate(js):
                r = j - 4 * g
                q0 = 128 * r if r >= 0 else 0
                for h in range(2):
                    nc.tensor.matmul(
                        otps_h[h][:, q0:G],
                        vaug[:, j, 2 * h : 2 * h + 2, :],
                        pt_h[h][:, idx, q0:G],
                        start=(j == 0),
                        stop=(j == n_j - 1),
                    )

        for jg in range(n_jg + 1):
            if jg < n_jg:
                pend_new = emit_s_exp(jg)
                # extra pops in the last slots cover the scalar drain at
                # the group boundary
                pop_units(3 if jg >= n_jg - 2 else 2)
            if pend is not None:
                emit_o(pend)
            pend = pend_new if jg < n_jg else None

        # normalize: otps_h[0] = [O_h0 (0:64) | l_h0 (64:128)],
        # otps_h[1] = [l_h1 (0:64) | O_h1 (64:128)]. The l gather uses
        # partition-shifted copies (proven on HW); recip and the O*rinv
        # mults are all same-partition-base (mults read PSUM in place).
        l_sb = norm_pool.tile([P, G], F32, tag="lsb", name=f"l{p}{g}")
        rinv = norm_pool.tile([P, G], F32, tag="rinv", name=f"r{p}{g}")
        # fine_norm: 128-col pieces so dependent proj chunks can start
        # after the first piece instead of the whole-group norm
        npc = 4 if fine_norm else 1
        w = G // npc
        for pc in range(npc):
            c0, c1 = w * pc, w * pc + w
            nc.vector.tensor_copy(l_sb[0:64, c0:c1], otps_h[0][64:128, c0:c1])
            nc.vector.tensor_copy(l_sb[64:128, c0:c1], otps_h[1][0:64, c0:c1])
            nc.vector.reciprocal_approx_fast(rinv[:, c0:c1], l_sb[:, c0:c1])
            nc.vector.tensor_tensor(
                ohat[0:64, G * g + c0 : G * g + c1],
                otps_h[0][0:64, c0:c1], rinv[0:64, c0:c1],
                mybir.AluOpType.mult,
            )
            nc.vector.tensor_tensor(
                ohat[64:128, G * g + c0 : G * g + c1],
                otps_h[1][64:128, c0:c1], rinv[64:128, c0:c1],
                mybir.AluOpType.mult,
            )
        pop_units(3)

    # ================= emission =================
    sts = {0: new_state(0), 1: new_state(1)}

    # attention interleave order; (1,3) before (0,3) so the g3 proj pair-1
    # half runs as filler during (0,3) and the tail is pair-0 only
    order = [(0, 0), (1, 0), (0, 1), (1, 1), (0, 2), (1, 2), (1, 3), (0, 3)]
    oidx = {pg: i for i, pg in enumerate(order)}

    # warmup: q/k chains first (wq/wk land first), then V tb0-3 (wv later)
    emit_qk_group(sts[0], "q", 0, "scalar")
    emit_qk_group(sts[0], "k", 0, "vector")
    emit_qk_group(sts[1], "q", 0, "scalar")
    emit_qk_group(sts[1], "k", 0, "vector")
    for tb in range(4):
        emit_v_block(tb)

    # queue fillers: forced at the order index where required, poppable
    # one group earlier (just-in-time, preserving late-group reserve).
    # q(p,tg) is read only by group (p,tg); k(p,tg) by every (p,g>=tg).
    def queue_qk(p, tg, which):
        if which == "q":
            req = oidx[(p, tg)]
        else:
            req = min(oidx[(p, g)] for g in range(tg, NG))
        units.append(
            (req, max(0, req - 1),
             lambda p=p, w=which, tg=tg: emit_qk_group(
                 sts[p], w, tg,
                 "scalar" if phase["exps_left"] > 56 else "vector"))
        )

    for tg in range(NG):
        for p in (0, 1):
            for which in ("q", "k"):
                if tg == 0:
                    continue  # emitted eagerly in warmup
                queue_qk(p, tg, which)
    for tb in range(4, NKB):
        g = tb // 4
        req = min(oidx[(0, g)], oidx[(1, g)])
        units.append((req, max(0, req - 1), lambda tb=tb: emit_v_block(tb)))
    units.sort(key=lambda u: u[0])

    for i, (p, g) in enumerate(order):
        cur_idx[0] = i
        force_units(i)
        emit_attn_g(sts[p], g, fine_norm=((p, g) == (0, 3)))
        done = {pg for pg in order[: i + 1]}
        if (0, g) in done and (1, g) in done and g < 3:
            # chained proj for g0-g2; two g2 chunks held to the drain to
            # bridge the (0,3)-norm -> g3-tail handoff at full clock
            for tc4 in range(G // P):
                rel = BIG if (g == 2 and tc4 >= 2) else i
                units.append(
                    (BIG, rel,
                     lambda g=g, tc4=tc4: emit_proj_chunk(g, tc4, "auto"))
                )
        if (p, g) == (1, 3):
            for tc4 in range(G // P):
                units.append((BIG, i, lambda tc4=tc4: emit_proj_g3_p1(tc4)))

    # tail: drain remaining fillers (incl. the two held g2 proj chunks),
    # then pair-0 g3 proj
    while units:
        _, _, fn = units.pop(0)
        fn()
    for tc4 in range(G // P):
        emit_proj_g3_p0(tc4)
    ctx.close()


def _build():
    if "nc" in _nc_cache:
        return _nc_cache["nc"]
    nc = bacc.Bacc("TRN2", target_bir_lowering=False, debug=False)
    with tile.TileContext(nc) as tc:
        _emit(tc)
    nc.compile()
    _nc_cache["nc"] = nc
    return nc


def _make_in_maps(x, wq, wk, wv, w_proj):
    import ml_dtypes

    bf16 = ml_dtypes.bfloat16
    xtb = [np.ascontiguousarray(x[b].T).astype(bf16) for b in range(B)]
    tri = np.triu(np.ones((P, P), dtype=np.float32)).astype(bf16)
    in_maps = []
    for c in range(NCORES):
        b, hg = c // 4, c % 4
        h0 = HPC * hg
        wq_cat = np.concatenate([wq[h0 + i] for i in range(HPC)], axis=1)
        wk_cat = np.concatenate([wk[h0 + i] for i in range(HPC)], axis=1)
        wv_cat = np.concatenate([wv[h0 + i] for i in range(HPC)], axis=1)
        wpt = w_proj[:, 256 * hg : 256 * (hg + 1)].T  # [256, C]
        in_maps.append(
            {
                "xt": xtb[b],
                "wq2": np.ascontiguousarray(
                    wq_cat.reshape(P, NPO, 256)).astype(bf16),
                "wk2": np.ascontiguousarray(
                    wk_cat.reshape(P, NPO, 256)).astype(bf16),
                "wv2": np.ascontiguousarray(
                    wv_cat.reshape(P, NPO, 256)).astype(bf16),
                "wpt": np.ascontiguousarray(
                    wpt.reshape(2, P, C).transpose(1, 0, 2)).astype(bf16),
                "tri": tri,
            }
        )
    return in_maps


def kernel(x, wq, wk, wv, w_proj, b_proj):
    x = np.asarray(x, dtype=np.float32)
    wq = np.asarray(wq, dtype=np.float32)
    wk = np.asarray(wk, dtype=np.float32)
    wv = np.asarray(wv, dtype=np.float32)
    w_proj = np.asarray(w_proj, dtype=np.float32)
    b_proj = np.asarray(b_proj, dtype=np.float32)

    nc = _build()
    in_maps = _make_in_maps(x, wq, wk, wv, w_proj)
    res = run_bass_kernel_spmd(nc, in_maps, core_ids=list(range(NCORES)))
    acc = np.zeros((B, T, C), dtype=np.float64)
    for c, r in enumerate(res.results):
        acc[c // 4] += np.asarray(r["out"], dtype=np.float64)
        acc[c // 4, T - G :] += np.asarray(r["outp1"], dtype=np.float64)
    return (acc + b_proj).astype(np.float32)
